# revision 18
# baseline (speedup 1.0000x reference)
"""Trainium2 Bass kernel for CLIP-style multi-head self-attention.

Problem shape: B=8, T=1024, D=1024, H=16 heads (head_dim 64), fp32 I/O.

Sharding: pure data parallel — one batch element per NeuronCore (8 cores),
no collectives. Each core computes the full attention layer for its batch
element; the host stacks the 8 outputs.

Per-core dataflow (all big matmuls in bf16 with fp32 PSUM accumulation):
  1. x -> bf16, PE-transpose to xT [D, T].
  2. QT = (Wq^T x^T)*scale, KT = Wk^T x^T   (lhsT = W chunks, rhs = xT)  [D, T]
     V  = x Wv (natural [T, D]) evicted into V_aug [T, H*(HD+1)] with a
     ones-column per head (gives softmax denominators for free).
  3. Per head pair (2 heads share a 128-partition tile of QT/KT):
     ST[Tk, Ti] = K Q^T via two row-tiled K=64 matmuls (tile_position 0/64),
     exp on ACT (scores are in [-4, 4] -> no max subtraction needed),
     A_aug[Ti, HD+1] = PT^T V_aug accumulated over Tk chunks.
     Normalize A rows by reciprocal of the ones-column sum (per-partition).
  4. PE-transpose A -> AT chunks, out = AT^T Wo + bo, DMA to DRAM.
"""

from contextlib import ExitStack

import numpy as np

import concourse.bass as bass
import concourse.tile as tile
from concourse import bacc
from concourse import mybir
from concourse.bass_utils import run_bass_kernel_spmd
from concourse.masks import make_identity

B, T, D, H = 8, 1024, 1024, 16
HD = D // H  # 64
SCALE = float(HD) ** -0.5
P = 128
NT = T // P  # 8
ND = D // P  # 8
FP32 = mybir.dt.float32
BF16 = mybir.dt.bfloat16
AF = mybir.ActivationFunctionType


def _build_attention(ctx, tc, hs, Wq, bq, Wk, bk, Wv, bv, Wo, bo, out):
    nc = tc.nc

    # ---------- persistent pools (allocated low, live for whole kernel) ----
    const_pool = ctx.enter_context(tc.tile_pool(name="const", bufs=1))
    qk_pool = ctx.enter_context(tc.tile_pool(name="qk", bufs=1))
    va_pool = ctx.enter_context(tc.tile_pool(name="va", bufs=1))
    wo_pool = ctx.enter_context(tc.tile_pool(name="wo", bufs=1))
    ps_small = ctx.enter_context(tc.tile_pool(name="ps_small", bufs=4, space="PSUM"))
    ps_big = ctx.enter_context(tc.tile_pool(name="ps_big", bufs=1, space="PSUM"))

    ident = const_pool.tile([P, P], BF16, name="ident")
    make_identity(nc, ident)
    zerob = const_pool.tile([P, 1], FP32, name="zerob")
    nc.vector.memset(zerob, 0.0)

    # biases laid out [P, ND]: column m holds bias[m*128 : (m+1)*128]
    bqs = const_pool.tile([P, ND], FP32, name="bqs")
    bks = const_pool.tile([P, ND], FP32, name="bks")
    nc.scalar.dma_start(out=bqs, in_=bq.rearrange("(n p) -> p n", p=P))
    nc.scalar.dma_start(out=bks, in_=bk.rearrange("(n p) -> p n", p=P))
    # fold the query scale into the bias (activation computes f(x*scale+bias))
    nc.vector.tensor_scalar_mul(bqs, bqs, SCALE)

    # bv/bo broadcast across partitions [P, D]
    bvb = const_pool.tile([P, D], FP32, name="bvb")
    bob = const_pool.tile([P, D], FP32, name="bob")
    nc.scalar.dma_start(
        out=bvb, in_=bass.AP(tensor=bv.tensor, offset=bv.offset, ap=[[0, P]] + bv.ap)
    )
    nc.scalar.dma_start(
        out=bob, in_=bass.AP(tensor=bo.tensor, offset=bo.offset, ap=[[0, P]] + bo.ap)
    )

    QT = [qk_pool.tile([P, T], BF16, name=f"qt{m}", tag=f"qt{m}") for m in range(ND)]
    KT = [qk_pool.tile([P, T], BF16, name=f"kt{m}", tag=f"kt{m}") for m in range(ND)]
    V_aug = [
        va_pool.tile([P, H * (HD + 1)], BF16, name=f"va{m}", tag=f"va{m}")
        for m in range(NT)
    ]
    Wo_bf = [
        wo_pool.tile([P, D], BF16, name=f"wob{k}", tag=f"wob{k}") for k in range(ND)
    ]

    # ---------- scoped pools: input staging + weights only needed early ----
    ident32 = const_pool.tile([P, P], FP32, name="ident32")
    make_identity(nc, ident32)

    with (
        tc.tile_pool(name="xstage", bufs=1) as xstage,
        tc.tile_pool(name="stage", bufs=1) as stage,
        tc.tile_pool(name="xt", bufs=1) as xt_pool,
        tc.tile_pool(name="w3", bufs=1) as w3_pool,
    ):
        # load x f32 (dedicated slots: a DMA never recycles another DMA's
        # slot here, keeping each descriptor at <=1 sync wait)
        x_f32 = []
        for t in range(NT):
            st_ = xstage.tile([P, D], FP32, name=f"xs{t}", tag=f"xs{t}")
            nc.scalar.dma_start(out=st_, in_=hs[t * P : (t + 1) * P, :])
            x_f32.append(st_)

        # PE-transpose x (f32) -> evict with cast -> xT [D, T] bf16
        xT = []
        for d in range(ND):
            xt = xt_pool.tile([P, T], BF16, name=f"xt{d}", tag=f"xt{d}")
            xT.append(xt)
            for j in range(2):
                ptr = ps_small.tile([P, 512], FP32, name=f"ptr{d}_{j}", tag="ps")
                for c in range(4):
                    t = j * 4 + c
                    nc.tensor.transpose(
                        ptr[:, c * P : (c + 1) * P],
                        x_f32[t][:, d * P : (d + 1) * P],
                        ident32,
                    )
                nc.vector.tensor_copy(xt[:, j * 512 : (j + 1) * 512], ptr)

        def load_w_bf(wap, pool, nm):
            # tag per k: slot reuse distance is exactly 8 DMAs, matching the
            # 8-queue HWDGE round-robin, so the WAW predecessor is FIFO-ordered
            # on the same queue and the descriptor needs only one sync wait
            tiles = []
            for k in range(ND):
                st_ = stage.tile([P, D], FP32, name=f"{nm}s{k}", tag=f"ws{k}")
                nc.scalar.dma_start(out=st_, in_=wap[k * P : (k + 1) * P, :])
                wb = pool.tile([P, D], BF16, name=f"{nm}{k}", tag=f"{nm}{k}")
                nc.scalar.copy(wb, st_)
                tiles.append(wb)
            return tiles

        Wq_bf = load_w_bf(Wq, w3_pool, "wqb")
        Wk_bf = load_w_bf(Wk, w3_pool, "wkb")
        Wv_bf = load_w_bf(Wv, w3_pool, "wvb")
        for k in range(ND):
            st_ = stage.tile([P, D], FP32, name=f"wos{k}", tag=f"ws{k}")
            nc.scalar.dma_start(out=st_, in_=Wo[k * P : (k + 1) * P, :])
            nc.scalar.copy(Wo_bf[k], st_)

        # V_aug ones columns: memset whole tile to 1.0, evictions overwrite
        for m in range(NT):
            nc.vector.memset(V_aug[m][:], 1.0)

        # ---------- projections ----------
        for m in range(ND):
            for j in range(2):
                jsl = slice(j * 512, (j + 1) * 512)
                msl = slice(m * P, (m + 1) * P)
                pq = ps_small.tile([P, 512], FP32, name=f"pq{m}_{j}", tag="ps")
                for k in range(ND):
                    nc.tensor.matmul(
                        pq,
                        Wq_bf[k][:, msl],
                        xT[k][:, jsl],
                        start=(k == 0),
                        stop=(k == ND - 1),
                    )
                nc.scalar.activation(
                    QT[m][:, jsl], pq, AF.Identity, bias=bqs[:, m : m + 1], scale=SCALE
                )

                pk = ps_small.tile([P, 512], FP32, name=f"pk{m}_{j}", tag="ps")
                for k in range(ND):
                    nc.tensor.matmul(
                        pk,
                        Wk_bf[k][:, msl],
                        xT[k][:, jsl],
                        start=(k == 0),
                        stop=(k == ND - 1),
                    )
                nc.scalar.activation(
                    KT[m][:, jsl], pk, AF.Identity, bias=bks[:, m : m + 1], scale=1.0
                )

                pv = ps_small.tile([P, 512], FP32, name=f"pv{m}_{j}", tag="ps")
                for k in range(ND):
                    nc.tensor.matmul(
                        pv,
                        xT[k][:, msl],
                        Wv_bf[k][:, jsl],
                        start=(k == 0),
                        stop=(k == ND - 1),
                    )
                vdst = V_aug[m][:].rearrange("p (h w) -> p h w", w=HD + 1)[
                    :, j * 8 : (j + 1) * 8, 0:HD
                ]
                nc.vector.tensor_add(
                    vdst,
                    pv[:].rearrange("p (h w) -> p h w", w=HD),
                    bvb[:, jsl].rearrange("p (h w) -> p h w", w=HD),
                )

    # ---------- attention (per pair of heads) ----------
    pt_pool = ctx.enter_context(tc.tile_pool(name="pt", bufs=16))
    rec_pool = ctx.enter_context(tc.tile_pool(name="rec", bufs=8))
    a_pool = ctx.enter_context(tc.tile_pool(name="apool", bufs=1))
    A_bf = [a_pool.tile([P, D], BF16, name=f"abf{i}", tag=f"abf{i}") for i in range(NT)]

    for p in range(H // 2):
        pts = []
        for kt in range(NT):
            st_ps = ps_big.tile([P, 2 * T], FP32, name=f"st{p}_{kt}", tag="st")
            for hi in range(2):
                base = hi * HD
                lhsT = KT[p][base : base + HD, kt * P : (kt + 1) * P]
                for j in range(2):
                    rhs = QT[p][base : base + HD, j * 512 : (j + 1) * 512]
                    nc.tensor.matmul(
                        st_ps[:, hi * T + j * 512 : hi * T + (j + 1) * 512],
                        lhsT,
                        rhs,
                        start=True,
                        stop=True,
                        tile_position=(base, 0),
                    )
            ptt = pt_pool.tile([P, 2 * T], BF16, name=f"pt{p}_{kt}", tag="pt")
            nc.scalar.activation(ptt, st_ps, AF.Exp, bias=zerob, scale=1.0)
            pts.append(ptt)

        for hi in range(2):
            h = 2 * p + hi
            for it in range(NT):
                pa = ps_small.tile([P, HD + 1], FP32, name=f"pa{h}_{it}", tag="ps")
                for kt in range(NT):
                    nc.tensor.matmul(
                        pa,
                        pts[kt][:, hi * T + it * P : hi * T + (it + 1) * P],
                        V_aug[kt][:, h * (HD + 1) : (h + 1) * (HD + 1)],
                        start=(kt == 0),
                        stop=(kt == NT - 1),
                    )
                rec = rec_pool.tile([P, 1], FP32, name=f"rec{h}_{it}", tag="rec")
                nc.vector.reciprocal(rec, pa[:, HD : HD + 1])
                nc.vector.tensor_scalar_mul(
                    A_bf[it][:, h * HD : (h + 1) * HD], pa[:, 0:HD], rec
                )

    # ---------- output projection ----------
    atc_pool = ctx.enter_context(tc.tile_pool(name="atc", bufs=4))
    osb_pool = ctx.enter_context(tc.tile_pool(name="osb", bufs=2))

    for it in range(NT):
        atcs = []
        for j in range(2):
            ptr = ps_small.tile([P, 512], BF16, name=f"patr{it}_{j}", tag="ps")
            for c in range(4):
                m = j * 4 + c
                nc.tensor.transpose(
                    ptr[:, c * P : (c + 1) * P],
                    A_bf[it][:, m * P : (m + 1) * P],
                    ident,
                )
            atc = atc_pool.tile([P, 512], BF16, name=f"atc{it}_{j}", tag="atc")
            nc.vector.tensor_copy(atc, ptr)
            atcs.append(atc)

        osb = osb_pool.tile([P, D], FP32, name=f"osb{it}", tag="osb")
        for j in range(2):
            jsl = slice(j * 512, (j + 1) * 512)
            po = ps_small.tile([P, 512], FP32, name=f"po{it}_{j}", tag="ps")
            for m in range(ND):
                nc.tensor.matmul(
                    po,
                    atcs[m // 4][:, (m % 4) * P : (m % 4 + 1) * P],
                    Wo_bf[m][:, jsl],
                    start=(m == 0),
                    stop=(m == ND - 1),
                )
            nc.vector.tensor_add(osb[:, jsl], po, bob[:, jsl])
        nc.scalar.dma_start(out=out[it * P : (it + 1) * P, :], in_=osb)


_CACHE = {}


def _get_nc():
    if "nc" not in _CACHE:
        nc = bacc.Bacc("TRN2", target_bir_lowering=False, debug=False, num_devices=B)
        io = {}
        io["hs"] = nc.dram_tensor("hs", [T, D], FP32, kind="ExternalInput").ap()
        for w in ("Wq", "Wk", "Wv", "Wo"):
            io[w] = nc.dram_tensor(w, [D, D], FP32, kind="ExternalInput").ap()
        for bias in ("bq", "bk", "bv", "bo"):
            io[bias] = nc.dram_tensor(bias, [D], FP32, kind="ExternalInput").ap()
        io["out"] = nc.dram_tensor("out", [T, D], FP32, kind="ExternalOutput").ap()
        with tile.TileContext(nc) as tc:
            with ExitStack() as ctx:
                _build_attention(
                    ctx,
                    tc,
                    io["hs"],
                    io["Wq"],
                    io["bq"],
                    io["Wk"],
                    io["bk"],
                    io["Wv"],
                    io["bv"],
                    io["Wo"],
                    io["bo"],
                    io["out"],
                )
        nc.compile()
        _CACHE["nc"] = nc
    return _CACHE["nc"]


def _in_maps(hidden_states, Wq, bq, Wk, bk, Wv, bv, Wo, bo):
    shared = {}
    for name, arr in (
        ("Wq", Wq), ("bq", bq), ("Wk", Wk), ("bk", bk),
        ("Wv", Wv), ("bv", bv), ("Wo", Wo), ("bo", bo),
    ):
        shared[name] = np.ascontiguousarray(np.asarray(arr), dtype=np.float32)
    hsf = np.ascontiguousarray(np.asarray(hidden_states), dtype=np.float32)
    return [dict(shared, hs=hsf[b]) for b in range(B)]


def kernel(hidden_states, Wq, bq, Wk, bk, Wv, bv, Wo, bo, **run_kwargs):
    nc = _get_nc()
    maps = _in_maps(hidden_states, Wq, bq, Wk, bk, Wv, bv, Wo, bo)
    res = run_bass_kernel_spmd(nc, maps, list(range(B)), **run_kwargs)
    out = np.stack([res.results[b]["out"] for b in range(B)], axis=0)
    if run_kwargs:
        _CACHE["last_results"] = res
    return out.astype(np.float32, copy=False)
